# revision 19
# baseline (speedup 1.0000x reference)
"""Trainium2 Bass kernel for nn_Attention (B=4, N=2048, D=1024, H=16, Hd=64).

Sharding: 8 cores = 4 batches x 2 head-groups. Core c handles batch c//2 and
heads [ (c%2)*8, (c%2)*8+8 ).  Each core computes qkv projections for its
heads, attention, and a partial output projection (contraction over its 512
head-dims of W_proj). Host sums the two partials per batch and adds b_proj.

Per-core kernel (all matmuls bf16 with fp32 PSUM accumulation):
  - qkT[f, t]  = sum_d Wqk[d, f] * xT[d, t]     (Q^T/K^T per head, [64, 2048])
  - v[t, f]    = sum_d xT[d, t] * Wv[d, f]       ([2048, 512], keys-major)
  - per head pair (2 heads packed in PE row/col groups):
      S^T[k, q] = sum_d K^T[d, k] Q^T[d, q]      (keys on partitions)
      E = exp(S^T / 8)   (ScalarE, bf16 out)
      U^T[hd, q] += sum_k V[k, hd] E[k, q]       (PSUM accumulate over key tiles)
      Eacc += E (VectorE);  sums = partition_all_reduce(Eacc)  (GpSimd)
      Uhat = U^T * (1/sums)                      (normalize during PSUM drain)
  - y[q, e] = sum_hd Uhat[hd, q] Wp[hd, e]       (partial; host adds pair+bias)
"""

import os
import sys
import types

import numpy as np

# --- environment bootstrap (grading env == dev env: axon-tunneled trn2) ----
for _p in ("/opt/trn_rl_repo", "/root/.axon_site/_ro/trn_rl_repo"):
    if _p not in sys.path and os.path.isdir(_p):
        sys.path.append(_p)

import ml_dtypes  # noqa: E402

BF16 = ml_dtypes.bfloat16


def _install_ntff_shim():
    """antenv.axon_hooks is missing on this image; provide it and register the
    ctypes NTFF hook so trace=True can report HW exec time."""
    if "antenv.axon_hooks" in sys.modules:
        return
    mod = types.ModuleType("antenv.axon_hooks")
    mod._hook = None
    mod.set_axon_ntff_profile_hook = lambda h: setattr(mod, "_hook", h)
    mod.get_axon_ntff_profile_hook = lambda: mod._hook
    sys.modules["antenv.axon_hooks"] = mod
    try:
        import antenv

        antenv.axon_hooks = mod
    except ImportError:
        pass
    try:
        from trn_agent_boot.trn_boot import _ntff_profile_via_ctypes

        hook = _ntff_profile_via_ctypes("/opt/axon/libaxon_pjrt.so")
        if hook is not None:
            mod.set_axon_ntff_profile_hook(hook)
    except Exception:
        pass


_install_ntff_shim()

import concourse.bacc as bacc  # noqa: E402
import concourse.bass as bass  # noqa: E402
import concourse.tile as tile  # noqa: E402
import concourse.bass_isa as bass_isa  # noqa: E402
from concourse import mybir  # noqa: E402
import concourse.bass_utils as bass_utils  # noqa: E402

# no S3 in the container; keep NTFF artifacts local
bass_utils.upload_artifacts = lambda tmpdir: tmpdir

F32 = mybir.dt.float32
BF = mybir.dt.bfloat16
EXP = mybir.ActivationFunctionType.Exp

N_CORES = 8
NT = 2048  # tokens
D = 1024  # d_model
NH_LOC = 8  # heads per core
HD = 64  # head dim
SCALE = HD**-0.5


def _body(tc: "tile.TileContext", ctx, y, xT, wqk, wv, wp):
    nc = tc.nc

    wpool = ctx.enter_context(tc.tile_pool(name="wpool", bufs=1))
    qkpool = ctx.enter_context(tc.tile_pool(name="qkpool", bufs=1))
    vpool = ctx.enter_context(tc.tile_pool(name="vpool", bufs=1))
    upool = ctx.enter_context(tc.tile_pool(name="upool", bufs=1))
    epool = ctx.enter_context(tc.tile_pool(name="epool", bufs=10))
    eaccpool = ctx.enter_context(tc.tile_pool(name="eaccpool", bufs=4))
    spool = ctx.enter_context(tc.tile_pool(name="spool", bufs=1))
    rpool = ctx.enter_context(tc.tile_pool(name="rpool", bufs=1))
    opool = ctx.enter_context(tc.tile_pool(name="opool", bufs=3))
    # PSUM budget (8 banks): scores 2x[128,1024] (4) + U^T/sums [128,1024]
    # (2) + filler pool 2x[128,512] (2). The filler pool decouples qkv/proj
    # background matmuls from the score/exp pipeline slots.
    psb = ctx.enter_context(tc.tile_pool(name="psb", bufs=2, space="PSUM"))
    psu = ctx.enter_context(tc.tile_pool(name="psu", bufs=1, space="PSUM"))
    pfill = ctx.enter_context(tc.tile_pool(name="pfill", bufs=2, space="PSUM"))

    # ---- persistent SBUF tensors -----------------------------------------
    xT_sb = [wpool.tile([128, NT], BF, tag=f"xT{i}", name=f"xT{i}") for i in range(8)]
    wqk_sb = [wpool.tile([128, 1024], BF, tag=f"wqk{i}", name=f"wqk{i}") for i in range(8)]
    wv_sb = [wpool.tile([128, 512], BF, tag=f"wv{i}", name=f"wv{i}") for i in range(8)]
    wp_sb = [wpool.tile([128, 1024], BF, tag=f"wp{i}", name=f"wp{i}") for i in range(4)]
    # need-ordered input DMAs: the lead-in chains consume x tokens 0:1024 and
    # the Q/K weight halves first; later token chunks and wp can trickle in.
    for i in range(8):
        nc.sync.dma_start(out=xT_sb[i][:, 0:512], in_=xT[i * 128:(i + 1) * 128, 0:512])
    for i in range(8):
        nc.sync.dma_start(out=wqk_sb[i][:, 0:512], in_=wqk[i * 128:(i + 1) * 128, 0:512])
    for i in range(8):
        nc.sync.dma_start(out=wqk_sb[i][:, 512:1024], in_=wqk[i * 128:(i + 1) * 128, 512:1024])
    for i in range(8):
        nc.sync.dma_start(out=xT_sb[i][:, 512:1024], in_=xT[i * 128:(i + 1) * 128, 512:1024])
    for i in range(8):
        nc.sync.dma_start(out=wv_sb[i], in_=wv[i * 128:(i + 1) * 128, :])
    for ts in (2, 3):
        for i in range(8):
            nc.sync.dma_start(out=xT_sb[i][:, ts * 512:(ts + 1) * 512],
                              in_=xT[i * 128:(i + 1) * 128, ts * 512:(ts + 1) * 512])
    for i in range(4):
        nc.sync.dma_start(out=wp_sb[i], in_=wp[i * 128:(i + 1) * 128, :])

    qkT = [qkpool.tile([128, NT], BF, tag=f"qkT{f}", name=f"qkT{f}") for f in range(8)]
    v_sb = [vpool.tile([128, 512], BF, tag=f"v{t}", name=f"v{t}") for t in range(16)]
    uhat = [upool.tile([128, NT], BF, tag=f"uh{p}", name=f"uh{p}") for p in range(4)]
    ones64 = wpool.tile([128, 64], BF, tag="ones64", name="ones64")
    nc.vector.memset(ones64, 1.0)

    # ---- background units (run on the filler PSUM pool) -------------------
    def qk_sub(f, ts2):
        # qkT[f][:, ts2*512:(ts2+1)*512] = (x @ Wqk[:, f-chunk]).T slice
        ps = pfill.tile([128, 512], F32, tag="pf", name=f"qk_ps{f}_{ts2}")
        for d in range(8):
            nc.tensor.matmul(
                ps[:, :],
                wqk_sb[d][:, f * 128 : (f + 1) * 128],
                xT_sb[d][:, ts2 * 512 : (ts2 + 1) * 512],
                start=(d == 0),
                stop=(d == 7),
            )
        nc.vector.tensor_copy(out=qkT[f][:, ts2 * 512 : (ts2 + 1) * 512], in_=ps[:])

    def v_unit(t):
        ps = pfill.tile([128, 512], F32, tag="pf", name=f"v_ps{t}")
        for d in range(8):
            nc.tensor.matmul(
                ps[:, :],
                xT_sb[d][:, t * 128 : (t + 1) * 128],
                wv_sb[d][:, :],
                start=(d == 0),
                stop=(d == 7),
            )
        nc.vector.tensor_copy(out=v_sb[t], in_=ps[:])

    def proj_sub(qt, es, pool=None, tag="pf"):
        # y[qt-tile, es-slice] partial over this core's 512 head dims
        pj = (pool or pfill).tile([128, 512], F32, tag=tag, name=f"pj{qt}_{es}")
        for c in range(4):
            nc.tensor.matmul(
                pj[:, :],
                uhat[c][:, qt * 128 : (qt + 1) * 128],
                wp_sb[c][:, es * 512 : (es + 1) * 512],
                start=(c == 0),
                stop=(c == 3),
            )
        ot = opool.tile([128, 512], F32, tag="out", name=f"ot{qt}_{es}")
        nc.vector.tensor_copy(out=ot, in_=pj[:])
        nc.sync.dma_start(
            out=y[qt * 128 : (qt + 1) * 128, es * 512 : (es + 1) * 512], in_=ot
        )

    # ---- attention for one pair of heads, one query half ------------------
    # `fillers`: background units woven one-per-kt-step into this pair's
    # stream. Every filler MUST be emitted before the first instruction that
    # consumes its output (in-order engine queues deadlock otherwise), so
    # each list is fully drained inside its own pair-half (15 slots >= len).
    def attention_pair_half(p, half, fillers=()):
        fillers = list(fillers)
        assert len(fillers) <= 15
        A, B = 2 * p, 2 * p + 1
        hsl = slice(half * 1024, (half + 1) * 1024)
        qA = qkT[p][0:64, hsl]
        qB = qkT[p][64:128, hsl]
        kA = qkT[4 + p][0:64, :]
        kB = qkT[4 + p][64:128, :]
        ut = psu.tile([128, 1024], F32, tag="ut", name=f"ut{p}_{half}")
        eaccAB = eaccpool.tile([128, 2048], BF, tag="eacc", name=f"eacc{p}_{half}")
        eABs = []

        def pv(ktpv, hb, s):
            head, r0, off = (A, 0, 0) if hb == 0 else (B, 64, 1024)
            e = eABs[ktpv]
            ssl = slice(s * 512, (s + 1) * 512)
            nc.tensor.matmul(
                ut[r0 : r0 + 64, ssl],
                v_sb[ktpv][:, head * 64 : (head + 1) * 64],
                e[:, off + s * 512 : off + (s + 1) * 512],
                start=(ktpv == 0),
                stop=(ktpv == 15),
            )

        def eacc_step(ktpv):
            if ktpv == 0:
                nc.vector.tensor_copy(out=eaccAB, in_=eABs[0])
            else:
                nc.vector.tensor_add(out=eaccAB, in0=eaccAB, in1=eABs[ktpv])

        for kt in range(16):
            ksl = slice(kt * 128, (kt + 1) * 128)
            eAB = epool.tile([128, 2048], BF, tag="e", name=f"e{p}_{half}_{kt}")
            eABs.append(eAB)
            # ready work (PV for kt-1, eacc, filler) goes BEFORE the QK score
            # groups: the in-order PE queue then reaches the QK slot-waits with
            # the previous exps already retired, instead of stalling on them.
            if kt > 1:
                pv(kt - 2, 0, 0)
                pv(kt - 2, 1, 0)
                pv(kt - 2, 0, 1)
                pv(kt - 2, 1, 1)
                eacc_step(kt - 2)
            if kt > 0 and fillers:
                fillers.pop(0)()
            # A/B interleaved: adjacent matmuls hit disjoint PE row groups,
            # so they overlap in the array (the pv(kt-2) lag guarantees both
            # score slots are already free when the PE reaches this block).
            stA = psb.tile([128, 1024], F32, tag="psb", name=f"stA{p}_{half}_{kt}")
            stB = psb.tile([128, 1024], F32, tag="psb", name=f"stB{p}_{half}_{kt}")
            for s in range(2):
                q0 = half * 1024 + s * 512
                nc.tensor.matmul(
                    stA[:, s * 512 : (s + 1) * 512], kA[:, ksl], qkT[p][0:64, q0 : q0 + 512],
                    start=True, stop=True,
                )
                nc.tensor.matmul(
                    stB[:, s * 512 : (s + 1) * 512], kB[:, ksl], qkT[p][64:128, q0 : q0 + 512],
                    start=True, stop=True,
                )
            nc.scalar.activation(out=eAB[:, 0:1024], in_=stA[:], func=EXP, scale=SCALE)
            nc.scalar.activation(out=eAB[:, 1024:2048], in_=stB[:], func=EXP, scale=SCALE)
        while fillers:
            fillers.pop(0)()
        for ktl in (14, 15):
            for s in range(2):
                pv(ktl, 0, s)
                pv(ktl, 1, s)
            eacc_step(ktl)
        # drain U^T (unnormalized) so the PSUM accumulator frees quickly
        nc.vector.tensor_copy(out=uhat[p][:, hsl], in_=ut[:])
        # softmax denominators for this half (ones-matmul -> spread-recip ->
        # partition_broadcast), then normalize in place
        sums_ps = psu.tile([128, 1024], F32, tag="ut", name=f"sums_ps{p}_{half}")
        for s in range(2):
            ssl = slice(s * 512, (s + 1) * 512)
            nc.tensor.matmul(sums_ps[0:64, ssl], ones64[:], eaccAB[:, s * 512:(s + 1) * 512], start=True, stop=True)
            nc.tensor.matmul(sums_ps[64:128, ssl], ones64[:], eaccAB[:, 1024 + s * 512:1024 + (s + 1) * 512], start=True, stop=True)
        sums = spool.tile([128, 1024], F32, tag="sums", name=f"sums{p}_{half}")
        nc.vector.tensor_copy(out=sums, in_=sums_ps[:])
        for hb in (0, 1):
            r0 = hb * 64
            rsp = spool.tile([128, 8], F32, tag="rsp", name=f"rsp{p}_{half}_{hb}")
            row = sums[r0 : r0 + 1, :].rearrange("p (a b) -> p a b", a=128)
            nc.gpsimd.dma_start(out=rsp[:], in_=row)
            rspr = spool.tile([128, 8], F32, tag="rspr", name=f"rspr{p}_{half}_{hb}")
            nc.vector.reciprocal(out=rspr[:], in_=rsp[:])
            rrow = spool.tile([1, 1024], F32, tag="rrow", bufs=1, name=f"rrow{p}_{half}_{hb}")
            nc.gpsimd.dma_start(
                out=rrow[0:1, :].rearrange("p (a b) -> p a b", a=128), in_=rspr[:]
            )
            rec = rpool.tile([128, 1024], F32, tag=f"rec{hb}", name=f"rec{p}_{half}_{hb}")
            nc.gpsimd.partition_broadcast(out_ap=rec[:, :], in_ap=rrow[0:1, :])
            nc.vector.tensor_mul(
                uhat[p][r0 : r0 + 64, hsl], uhat[p][r0 : r0 + 64, hsl], rec[r0 : r0 + 64, :]
            )

    # ---- schedule ---------------------------------------------------------
    # lead-in: q/k features for pair 0 plus the first v tiles; the rest of
    # the qkv projections and half-0's output projection weave into the
    # attention stream as per-pair filler lists (dependency-safe: each list
    # drains before the pair that consumes its outputs starts).
    def mk(fn, *args):
        return lambda: fn(*args)

    # minimal lead: pair-0 half-0 needs only Q half-0 (f0 ts0/1) and the
    # first key quarter (f4 ts0); the rest of f4 weaves in ahead of its kt
    # deadlines (f4tsX covers keys for kt in [4X, 4X+4), used at step 4X).
    qk_sub(0, 0)
    qk_sub(0, 1)
    qk_sub(4, 0)
    for t in range(7):
        v_unit(t)
    half0_fills = [
        [mk(v_unit, 7), mk(qk_sub, 4, 1), mk(v_unit, 8), mk(v_unit, 9),
         mk(qk_sub, 4, 2), mk(v_unit, 10), mk(v_unit, 11), mk(qk_sub, 4, 3),
         mk(v_unit, 12), mk(v_unit, 13), mk(v_unit, 14), mk(v_unit, 15),
         mk(qk_sub, 1, 0), mk(qk_sub, 1, 1), mk(qk_sub, 5, 0)],
        [mk(qk_sub, 5, 1), mk(qk_sub, 5, 2), mk(qk_sub, 5, 3),
         mk(qk_sub, 0, 2), mk(qk_sub, 0, 3), mk(qk_sub, 1, 2), mk(qk_sub, 1, 3)]
        + [mk(qk_sub, f, ts2) for f in (2, 6) for ts2 in range(4)],
        [mk(qk_sub, f, ts2) for f in (3, 7) for ts2 in range(4)],
        [],
    ]
    for p in range(4):
        attention_pair_half(p, 0, half0_fills[p])
    half1_fills = [
        [mk(proj_sub, qt, es) for qt in range(0, 2) for es in range(2)],
        [mk(proj_sub, qt, es) for qt in range(2, 4) for es in range(2)],
        [mk(proj_sub, qt, es) for qt in range(4, 6) for es in range(2)],
        [mk(proj_sub, qt, es) for qt in range(6, 8) for es in range(2)],
    ]
    for p in range(4):
        attention_pair_half(p, 1, half1_fills[p])
    for qt in range(8, 16):
        for es in range(2):
            if (qt * 2 + es) % 2 == 0:
                proj_sub(qt, es)
            else:
                proj_sub(qt, es, pool=psb, tag="psb")


_NC_CACHE = {}


def _build_nc():
    if "nc" in _NC_CACHE:
        return _NC_CACHE["nc"]
    nc = bacc.Bacc("TRN2", target_bir_lowering=False, debug=False, num_devices=N_CORES)
    xT = nc.dram_tensor("xT", [D, NT], BF, kind="ExternalInput").ap()
    wqk = nc.dram_tensor("wqk", [D, 1024], BF, kind="ExternalInput").ap()
    wv = nc.dram_tensor("wv", [D, 512], BF, kind="ExternalInput").ap()
    wp = nc.dram_tensor("wp", [512, 1024], BF, kind="ExternalInput").ap()
    y = nc.dram_tensor("y", [NT, 1024], F32, kind="ExternalOutput").ap()
    from contextlib import ExitStack

    with tile.TileContext(nc) as tc, ExitStack() as ctx:
        _body(tc, ctx, y, xT, wqk, wv, wp)
    nc.compile()
    _NC_CACHE["nc"] = nc
    return nc


def _prepare_in_maps(x, W_qkv, W_proj):
    x = np.asarray(x, dtype=np.float32)
    W_qkv = np.asarray(W_qkv, dtype=np.float32)
    W_proj = np.asarray(W_proj, dtype=np.float32)
    in_maps = []
    for c in range(N_CORES):
        b, hg = divmod(c, 2)
        cs = slice(hg * 512, (hg + 1) * 512)
        xTc = np.ascontiguousarray(x[b].T).astype(BF16)
        wqk = np.ascontiguousarray(
            np.concatenate([W_qkv[:, 0:1024][:, cs], W_qkv[:, 1024:2048][:, cs]], axis=1)
        ).astype(BF16)
        wv = np.ascontiguousarray(W_qkv[:, 2048:3072][:, cs]).astype(BF16)
        wp = np.ascontiguousarray(W_proj[cs, :]).astype(BF16)
        in_maps.append({"xT": xTc, "wqk": wqk, "wv": wv, "wp": wp})
    return in_maps


def _run(x, W_qkv, W_proj, b_proj, trace=False):
    nc = _build_nc()
    in_maps = _prepare_in_maps(x, W_qkv, W_proj)
    res = bass_utils.run_bass_kernel_spmd(
        nc, in_maps, core_ids=list(range(N_CORES)), trace=trace
    )
    b_proj = np.asarray(b_proj, dtype=np.float32)
    y = np.empty((4, NT, D), dtype=np.float32)
    for b in range(4):
        y[b] = res.results[2 * b]["y"] + res.results[2 * b + 1]["y"] + b_proj[None, :]
    return y, res


def kernel(x, W_qkv, W_proj, b_proj):
    y, _ = _run(x, W_qkv, W_proj, b_proj, trace=False)
    return y

